# revision 1
# baseline (speedup 1.0000x reference)
"""Trainium2 Bass kernel for nn_ADDMeanM_16595753632500.

out[b] = expm(D_b logm(X_b) D_b), X_b = f[b,0] (64x64 SPD), D_b = diag(w[b]),
B = 8192, data-parallel across 8 NeuronCores (1024 samples each).

Eigh-free algorithm (batched 64x64 matmuls only):
  tuned coupled Newton-Schulz sqrt chain (2 levels, 5+4 iters):
      Y ~ Xs^(1/4), Z ~ Xs^(-1/4), Xs = X/C
  S = (Y/r0 - r0 Z)/2 = sinh(log(Y/r0)); H = asinh(S) by odd series
  (10 terms, Paterson-Stockmeyer base S^6)
  A = w w^T o (H + GAMMA I); out = expm(A)^16 (Taylor-6 PS + 4 squarings)

Layout: 2 samples per 128 partitions (quadrant matmuls, base partitions
0/64), GANG=8 pairs side-by-side in the free dim so vector-engine ops
amortize across 16 samples (512-col ops). Gangs are emitted sequentially
(INTERLEAVE=1); cross-gang overlap comes from Tile slot rotation.
Measured: ~1.5-2.1 ms for all 8192 samples on 8 cores, rel err 3.8e-6.
"""
import os
import numpy as np

BUFS_WORK = int(os.environ.get("K_BUFS_WORK", "3"))
BUFS_PSA = int(os.environ.get("K_BUFS_PSA", "4"))
BUFS_PSB = int(os.environ.get("K_BUFS_PSB", "2"))
INTERLEAVE = int(os.environ.get("K_INTERLEAVE", "1"))
GANG = int(os.environ.get("K_GANG", "8"))
BLOCKDIAG = int(os.environ.get("K_BLOCKDIAG", "0"))

# ---------------- schedule constants (from offline tuning) ----------------
C = 6.4
R0 = 0.5931242054624994
GAMMA = -0.014569237901484997
LEV0 = [(4.463349828852388, -3.982928840755367),
        (1.5346492142150907, -0.3329187232555637),
        (1.5067541795842014, -0.4664705901173269),
        (1.5002144253897574, -0.4989281182106656),
        (1.5000002071882226, -0.49999896405907457)]
LEV1 = [(2.5754096741291352, -1.75518464610241),
        (1.504075781853448, -0.4797092884914813),
        (1.5000767445527003, -0.49961630865168327),
        (1.50000002651696, -0.4999998674156874)]
ASINH_CF = [0.25, -0.041666666666666664, 0.01875, -0.011160714285714286,
            0.007595486111111111, -0.005593039772727273,
            0.004338191105769231, -0.0034912109375,
            0.0028879502240349263, -0.0024404023822985196]
EXP_C = [1.0, 1.0, 0.5, 0.16666666666666666, 0.041666666666666664,
         0.008333333333333333, 0.001388888888888889]
N_CORES = 8
B_TOTAL = 8192
SHARD = B_TOTAL // N_CORES
NPAIR = SHARD // 2                  # 512
NGANG = NPAIR // GANG
N = 64
GW = GANG * N

# const blocks (each GW wide), in order:
#  0 cI(1.0), 1 aL0f(a1_0), 2 aL1f(a1_1),
#  3..6  a of LEV0[1..4], 7..9 a of LEV1[1..3],
#  10 cf0, 11 cf3, 12 cf6, 13 ec3
_CONST_VALS = ([1.0, LEV0[0][0], LEV1[0][0]]
               + [a for (a, b) in LEV0[1:]] + [a for (a, b) in LEV1[1:]]
               + [ASINH_CF[0], ASINH_CF[3], ASINH_CF[6], EXP_C[3]]
               + [ASINH_CF[2], ASINH_CF[4], ASINH_CF[8], EXP_C[2], EXP_C[4]])
NCONST = len(_CONST_VALS)


def _host_constants():
    eye = np.eye(N, dtype=np.float32)
    blk = np.zeros((128, NCONST * GW), np.float32)
    for k, v in enumerate(_CONST_VALS):
        for j in range(GANG):
            for t in range(2):
                blk[64 * t:64 * t + 64, k * GW + j * N:k * GW + (j + 1) * N] = v * eye
    return blk


def _host_weights(w_core):
    ws = w_core.reshape(NGANG, GANG, 2, N)
    wcol = np.ascontiguousarray(ws.transpose(0, 2, 3, 1)).reshape(NGANG, 128, GANG)
    wrep = np.broadcast_to(ws[:, :, :, None, :], (NGANG, GANG, 2, N, N))
    wrep = np.ascontiguousarray(wrep.transpose(0, 2, 3, 1, 4)).reshape(NGANG, 128, GANG * N)
    return np.ascontiguousarray(wcol), np.ascontiguousarray(wrep)


def build_nc(ngang=NGANG):
    import concourse.bacc as bacc
    import concourse.mybir as mybir
    import concourse.tile as tile

    dt = mybir.dt.float32
    n_samples = ngang * GANG * 2
    nc = bacc.Bacc()
    f_in = nc.declare_dram_parameter("f", [n_samples, N, N], dt, isOutput=False)
    wcol_in = nc.declare_dram_parameter("wcol", [ngang, 128, GANG], dt, isOutput=False)
    wrep_in = nc.declare_dram_parameter("wrep", [ngang, 128, GW], dt, isOutput=False)
    cst_in = nc.declare_dram_parameter("cst", [128, NCONST * GW], dt, isOutput=False)
    out_d = nc.declare_dram_parameter("out", [n_samples, N, N], dt, isOutput=True)

    f_gv = f_in[:].rearrange("(g j two) r c -> g j (two r) c", j=GANG, two=2)
    o_gv = out_d[:].rearrange("(g j two) r c -> g j (two r) c", j=GANG, two=2)

    mult = mybir.AluOpType.mult
    add = mybir.AluOpType.add
    sub = mybir.AluOpType.subtract
    CopyF = mybir.ActivationFunctionType.Copy

    with tile.TileContext(nc) as tc:
        with (
            tc.tile_pool(name="consts", bufs=1) as cpool,
            tc.tile_pool(name="work", bufs=BUFS_WORK) as wpool,
            tc.tile_pool(name="psA", bufs=BUFS_PSA, space="PSUM") as psA,
            tc.tile_pool(name="psB", bufs=BUFS_PSB, space="PSUM") as psB,
        ):
            cst = cpool.tile([128, NCONST * GW], dt)
            nc.sync.dma_start(cst[:], cst_in[:])

            def cblk(k):
                return cst[:, k * GW:(k + 1) * GW]
            cI = cblk(0)

            def gang_stages(gi):
                fg = wpool.tile([128, GW], dt, tag="fg")
                yz = wpool.tile([128, 2 * GW], dt, tag="yz")
                wt = wpool.tile([128, GW], dt, tag="wt")
                og = wpool.tile([128, GW], dt, tag="og")
                xs = wpool.tile([128, GW], dt, tag="xs")
                wcolt = wpool.tile([128, GANG], dt, tag="wcolt")
                wrept = wpool.tile([128, GW], dt, tag="wrept")

                for j in range(GANG):
                    nc.sync.dma_start(fg[:, j * N:(j + 1) * N], f_gv[gi, j])
                nc.sync.dma_start(wcolt[:], wcol_in[gi])
                nc.sync.dma_start(wrept[:], wrep_in[gi])
                yield

                yzv = yz[:].rearrange("p (j d) -> p j d", d=2 * N)
                Yv = yzv[:, :, 0:N]
                Zv = yzv[:, :, N:2 * N]

                def Ysl(j):
                    return yz[:, 2 * N * j: 2 * N * j + N]

                def Zsl(j):
                    return yz[:, 2 * N * j + N: 2 * N * j + 2 * N]

                def YZsl(j):
                    return yz[:, 2 * N * j: 2 * N * (j + 1)]

                def mm2(out_ap, lhsT_ap, rhs_ap):
                    for t in range(2):
                        ps = slice(64 * t, 64 * t + 64)
                        nc.tensor.matmul(out_ap[ps], lhsT_ap[ps], rhs_ap[ps])

                def sl(tile_, j):
                    return tile_[:, j * N:(j + 1) * N]

                # ---- level 0 first iteration: W = a1 I + (b1/C) X ----
                nc.vector.scalar_tensor_tensor(
                    wt[:], fg[:], float(LEV0[0][1] / C), cblk(1), mult, add)
                pT = psA.tile([128, GW], dt, tag="pT")
                for j in range(GANG):
                    mm2(sl(pT, j), sl(wt, j), sl(fg, j))
                # Y_1 = pT/C (strided dst); Z_1 = W (plain copy, gpsimd)
                nc.scalar.activation(Yv, pT[:].rearrange("p (j d) -> p j d", d=N),
                                     CopyF, scale=float(1.0 / C))
                nc.gpsimd.tensor_copy(Zv, wt[:].rearrange("p (j d) -> p j d", d=N))
                yield

                # ---- NS full iterations ----
                def ns_iter(lev, k, cb, last=None):
                    b = lev[k][1]
                    pT = psA.tile([128, GW], dt, tag="pT")
                    for j in range(GANG):
                        mm2(sl(pT, j), Zsl(j), Ysl(j))
                    nc.vector.scalar_tensor_tensor(
                        wt[:], pT[:], float(b), cblk(cb), mult, add)
                    if last is None and k + 1 < len(lev):
                        pYZ = psB.tile([128, 2 * GW], dt, tag="pYZ")
                        for j in range(GANG):
                            mm2(pYZ[:, 2 * N * j:2 * N * (j + 1)], sl(wt, j), YZsl(j))
                        nc.scalar.activation(yz[:], pYZ[:], CopyF)
                    elif last is None:
                        pT2 = psA.tile([128, GW], dt, tag="pT")
                        for j in range(GANG):
                            mm2(sl(pT2, j), sl(wt, j), Ysl(j))
                        nc.scalar.activation(Yv, pT2[:].rearrange(
                            "p (j d) -> p j d", d=N), CopyF)
                    else:
                        pYZ = psB.tile([128, 2 * GW], dt, tag="pYZ")
                        for j in range(GANG):
                            mm2(pYZ[:, 2 * N * j:2 * N * (j + 1)], sl(wt, j), YZsl(j))
                        pYZv = pYZ[:].rearrange("p (j d) -> p j d", d=2 * N)
                        nc.scalar.activation(Yv, pYZv[:, :, 0:N], CopyF,
                                             scale=float(last[0]))
                        nc.scalar.activation(Zv, pYZv[:, :, N:2 * N], CopyF,
                                             scale=float(last[1]))

                for k in range(1, len(LEV0)):
                    ns_iter(LEV0, k, 2 + k)
                    yield

                # ---- level 1 first iteration ----
                nc.vector.scalar_tensor_tensor(
                    wt[:].rearrange("p (j d) -> p j d", d=N), Yv,
                    float(LEV1[0][1]),
                    cblk(2)[:].rearrange("p (j d) -> p j d", d=N), mult, add)
                pT = psA.tile([128, GW], dt, tag="pT")
                for j in range(GANG):
                    mm2(sl(pT, j), sl(wt, j), Ysl(j))
                nc.gpsimd.tensor_copy(Zv, wt[:].rearrange("p (j d) -> p j d", d=N))
                nc.scalar.activation(Yv, pT[:].rearrange("p (j d) -> p j d", d=N),
                                     CopyF)
                yield

                for k in range(1, len(LEV1) - 1):
                    ns_iter(LEV1, k, 6 + k)
                    yield
                ns_iter(LEV1, len(LEV1) - 1, 6 + len(LEV1) - 1,
                        last=(0.5 / R0, 0.5 * R0))
                # S = Ys - Zs -> og
                nc.vector.tensor_tensor(og[:].rearrange("p (j d) -> p j d", d=N),
                                        Yv, Zv, sub)
                yield

                # ---- asinh PS series ----
                cf = ASINH_CF
                pT = psA.tile([128, GW], dt, tag="pT")
                for j in range(GANG):
                    mm2(sl(pT, j), sl(og, j), sl(og, j))        # S2
                nc.scalar.activation(wt[:], pT[:], CopyF)       # S2 -> wt
                yield
                pT = psA.tile([128, GW], dt, tag="pT")
                for j in range(GANG):
                    mm2(sl(pT, j), sl(wt, j), sl(wt, j))        # S4
                nc.scalar.activation(fg[:], pT[:], CopyF)       # S4 -> fg
                pT2 = psA.tile([128, GW], dt, tag="pT")
                for j in range(GANG):
                    mm2(sl(pT2, j), sl(fg, j), sl(wt, j))       # S6 = S4@S2
                nc.scalar.activation(xs[:], pT2[:], CopyF)      # S6 -> xs
                yield

                u = yz[:, 0:GW]
                v = yz[:, GW:2 * GW]
                # G2 = cf6 I + cf7 S2 + cf8 S4 ; P = G2 + cf9 S6 -> v
                nc.vector.scalar_tensor_tensor(u, wt[:], float(cf[7]), cblk(12),
                                               mult, add)
                nc.vector.scalar_tensor_tensor(u, fg[:], float(cf[8]), u, mult, add)
                nc.vector.scalar_tensor_tensor(v, xs[:], float(cf[9]), u, mult, add)
                pQ = psA.tile([128, GW], dt, tag="pT")
                for j in range(GANG):
                    mm2(sl(pQ, j), sl(xs, j), sl(v, j))         # S6 @ P
                nc.vector.scalar_tensor_tensor(u, wt[:], float(cf[4]), cblk(11),
                                               mult, add)
                nc.vector.scalar_tensor_tensor(u, fg[:], float(cf[5]), u, mult, add)
                nc.vector.tensor_tensor(v, pQ[:], u, add)       # P = G1 + S6P
                yield
                pQ = psA.tile([128, GW], dt, tag="pT")
                for j in range(GANG):
                    mm2(sl(pQ, j), sl(xs, j), sl(v, j))
                nc.vector.scalar_tensor_tensor(u, wt[:], float(cf[1]), cblk(10),
                                               mult, add)
                nc.vector.scalar_tensor_tensor(u, fg[:], float(cf[2]), u, mult, add)
                nc.vector.tensor_tensor(v, pQ[:], u, add)       # P = G0 + S6P
                pH = psA.tile([128, GW], dt, tag="pT")
                for j in range(GANG):
                    mm2(sl(pH, j), sl(og, j), sl(v, j))         # H = S @ P
                # A = w w^T o (H + GAMMA I) -> fg
                nc.vector.scalar_tensor_tensor(wt[:], cI, float(GAMMA), pH[:],
                                               mult, add)
                for j in range(GANG):
                    nc.vector.scalar_tensor_tensor(
                        sl(fg, j), sl(wt, j), wcolt[:, j:j + 1], sl(wrept, j),
                        mult, mult)
                yield

                # ---- exp Taylor-6, PS base A3 ----
                ec = EXP_C
                pT = psA.tile([128, GW], dt, tag="pT")
                for j in range(GANG):
                    mm2(sl(pT, j), sl(fg, j), sl(fg, j))        # A2
                nc.scalar.activation(wt[:], pT[:], CopyF)       # A2 -> wt
                pT2 = psA.tile([128, GW], dt, tag="pT")
                for j in range(GANG):
                    mm2(sl(pT2, j), sl(wt, j), sl(fg, j))       # A3 = A2@A
                nc.scalar.activation(xs[:], pT2[:], CopyF)      # A3 -> xs
                yield
                # P = (ec3 I + ec4 A + ec5 A2) + ec6 A3 -> v
                nc.vector.scalar_tensor_tensor(u, fg[:], float(ec[4]), cblk(13),
                                               mult, add)
                nc.vector.scalar_tensor_tensor(u, wt[:], float(ec[5]), u, mult, add)
                nc.vector.scalar_tensor_tensor(v, xs[:], float(ec[6]), u, mult, add)
                pG = psA.tile([128, GW], dt, tag="pT")
                for j in range(GANG):
                    mm2(sl(pG, j), sl(xs, j), sl(v, j))         # A3 @ P
                # G0 = I + A + ec2 A2 ; Gx = G0 + A3P -> og
                nc.vector.scalar_tensor_tensor(u, fg[:], float(ec[1]), cI, mult, add)
                nc.vector.scalar_tensor_tensor(u, wt[:], float(ec[2]), u, mult, add)
                nc.vector.tensor_tensor(og[:], pG[:], u, add)
                yield

                # ---- 4 squarings: og -> u -> v -> u -> og ----
                chain = [og[:], u, v, u, og[:]]
                for sq in range(4):
                    src, dst = chain[sq], chain[sq + 1]
                    pP = psA.tile([128, GW], dt, tag="pT")
                    for j in range(GANG):
                        mm2(pP[:, j * N:(j + 1) * N],
                            src[:, j * N:(j + 1) * N],
                            src[:, j * N:(j + 1) * N])
                    nc.scalar.activation(dst, pP[:], CopyF)
                    if sq in (1, 3):
                        yield
                for j in range(GANG):
                    nc.sync.dma_start(o_gv[gi, j], og[:, j * N:(j + 1) * N])

            def gang_stages_bd(gi):
                """Block-diagonal stationaries: one 128x128 lhsT per pair,
                halving PE streaming cycles (fp32 4-pass is per moving row)."""
                fg = wpool.tile([128, GW], dt, tag="fg")        # X / S2stk / Astk
                yz = wpool.tile([128, 2 * GW], dt, tag="yz")    # Y,Z stacked; later u,v
                wbd = wpool.tile([128, 2 * GW], dt, tag="wbd")  # W-BD / S-BD / G-BD
                zbd = wpool.tile([128, 2 * GW], dt, tag="zbd")  # Z-BD / S2-BD / A-BD
                xbd = wpool.tile([128, 2 * GW], dt, tag="xbd")  # S4-BD / A2-BD
                og = wpool.tile([128, GW], dt, tag="og")        # S / A2stk / G stk
                gtmp = wpool.tile([128, GW], dt, tag="gtmp")    # ping-pong G
                wcolt = wpool.tile([128, GANG], dt, tag="wcolt")
                wrept = wpool.tile([128, GW], dt, tag="wrept")

                for j in range(GANG):
                    nc.sync.dma_start(fg[:, j * N:(j + 1) * N], f_gv[gi, j])
                nc.sync.dma_start(wcolt[:], wcol_in[gi])
                nc.sync.dma_start(wrept[:], wrep_in[gi])
                nc.gpsimd.memset(wbd[:], 0.0)
                nc.gpsimd.memset(zbd[:], 0.0)
                nc.gpsimd.memset(xbd[:], 0.0)
                yield

                yzv = yz[:].rearrange("p (j d) -> p j d", d=2 * N)
                Yv = yzv[:, :, 0:N]
                Zv = yzv[:, :, N:2 * N]

                def Ysl(j):
                    return yz[:, 2 * N * j: 2 * N * j + N]

                def Zsl(j):
                    return yz[:, 2 * N * j + N: 2 * N * j + 2 * N]

                def YZsl(j):
                    return yz[:, 2 * N * j: 2 * N * (j + 1)]

                def bd(tile_, j):
                    return tile_[:, 2 * N * j: 2 * N * (j + 1)]

                def sl(tile_, j):
                    return tile_[:, j * N:(j + 1) * N]

                def halves(bdtile):
                    v = bdtile[:].rearrange("p (j d) -> p j d", d=2 * N)
                    return v[0:64, :, 0:N], v[64:128, :, N:2 * N]

                def stk_halves(t):
                    v = t[:].rearrange("p (j d) -> p j d", d=N)
                    return v[0:64], v[64:128]

                def stt_bd(dst_bd, in0_stk, scalar, cb):
                    """dst_bd diag blocks = in0_stk*scalar + const (2 half ops)"""
                    dt_, db_ = halves(dst_bd)
                    i0t, i0b = stk_halves(in0_stk)
                    ct, cb_ = stk_halves(cblk(cb))
                    nc.vector.scalar_tensor_tensor(dt_, i0t, float(scalar), ct,
                                                   mult, add)
                    nc.vector.scalar_tensor_tensor(db_, i0b, float(scalar), cb_,
                                                   mult, add)

                def bdify(dst_bd, src_stk):
                    """gpsimd copy stacked -> BD diag blocks (2 half ops)"""
                    dt_, db_ = halves(dst_bd)
                    st, sb = stk_halves(src_stk)
                    nc.gpsimd.tensor_copy(dt_, st)
                    nc.gpsimd.tensor_copy(db_, sb)

                # ---- level 0 first iteration ----
                stt_bd(wbd, fg, LEV0[0][1] / C, 1)
                pT = psA.tile([128, GW], dt, tag="pT")
                for j in range(GANG):
                    nc.tensor.matmul(sl(pT, j), bd(wbd, j), sl(fg, j))
                nc.scalar.activation(Yv, pT[:].rearrange("p (j d) -> p j d", d=N),
                                     CopyF, scale=float(1.0 / C))
                nc.gpsimd.tensor_copy(zbd[:], wbd[:])        # Z1-BD = W (BD->BD)
                # Z1 stacked:
                wv = wbd[:].rearrange("p (j d) -> p j d", d=2 * N)
                nc.gpsimd.tensor_copy(Zv[0:64], wv[0:64, :, 0:N])
                nc.gpsimd.tensor_copy(Zv[64:128], wv[64:128, :, N:2 * N])
                yield

                # ---- NS full iterations ----
                def ns_iter_bd(lev, k, cb, last=None):
                    b = lev[k][1]
                    pT = psA.tile([128, GW], dt, tag="pT")
                    for j in range(GANG):
                        nc.tensor.matmul(sl(pT, j), bd(zbd, j), Ysl(j))
                    stt_bd(wbd, pT, b, cb)
                    if last is None and k + 1 < len(lev):
                        pYZ = psB.tile([128, 2 * GW], dt, tag="pYZ")
                        for j in range(GANG):
                            nc.tensor.matmul(pYZ[:, 2 * N * j:2 * N * (j + 1)],
                                             bd(wbd, j), YZsl(j))
                        nc.scalar.activation(yz[:], pYZ[:], CopyF)
                        bdify(zbd, pYZ[:].rearrange("p (j d) -> p j d", d=2 * N)
                              [:, :, N:2 * N]) if False else None
                        # Z-BD from the freshly copied stacked Z
                        zt, zb_ = halves(zbd)
                        nc.gpsimd.tensor_copy(zt, Zv[0:64])
                        nc.gpsimd.tensor_copy(zb_, Zv[64:128])
                    elif last is None:
                        pT2 = psA.tile([128, GW], dt, tag="pT")
                        for j in range(GANG):
                            nc.tensor.matmul(sl(pT2, j), bd(wbd, j), Ysl(j))
                        nc.scalar.activation(Yv, pT2[:].rearrange(
                            "p (j d) -> p j d", d=N), CopyF)
                    else:
                        pYZ = psB.tile([128, 2 * GW], dt, tag="pYZ")
                        for j in range(GANG):
                            nc.tensor.matmul(pYZ[:, 2 * N * j:2 * N * (j + 1)],
                                             bd(wbd, j), YZsl(j))
                        pYZv = pYZ[:].rearrange("p (j d) -> p j d", d=2 * N)
                        nc.scalar.activation(Yv, pYZv[:, :, 0:N], CopyF,
                                             scale=float(last[0]))
                        nc.scalar.activation(Zv, pYZv[:, :, N:2 * N], CopyF,
                                             scale=float(last[1]))

                for k in range(1, len(LEV0)):
                    ns_iter_bd(LEV0, k, 2 + k)
                    yield

                # ---- level 1 first iteration ----
                wt_, wb_ = halves(wbd)
                ct, cb_ = stk_halves(cblk(2))
                nc.vector.scalar_tensor_tensor(wt_, Yv[0:64], float(LEV1[0][1]),
                                               ct, mult, add)
                nc.vector.scalar_tensor_tensor(wb_, Yv[64:128], float(LEV1[0][1]),
                                               cb_, mult, add)
                pT = psA.tile([128, GW], dt, tag="pT")
                for j in range(GANG):
                    nc.tensor.matmul(sl(pT, j), bd(wbd, j), Ysl(j))
                nc.gpsimd.tensor_copy(zbd[:], wbd[:])
                wv = wbd[:].rearrange("p (j d) -> p j d", d=2 * N)
                nc.gpsimd.tensor_copy(Zv[0:64], wv[0:64, :, 0:N])
                nc.gpsimd.tensor_copy(Zv[64:128], wv[64:128, :, N:2 * N])
                nc.scalar.activation(Yv, pT[:].rearrange("p (j d) -> p j d", d=N),
                                     CopyF)
                yield

                for k in range(1, len(LEV1) - 1):
                    ns_iter_bd(LEV1, k, 6 + k)
                    yield
                ns_iter_bd(LEV1, len(LEV1) - 1, 6 + len(LEV1) - 1,
                           last=(0.5 / R0, 0.5 * R0))
                # S = Ys - Zs -> og (stacked) ; S-BD -> wbd
                nc.vector.tensor_tensor(og[:].rearrange("p (j d) -> p j d", d=N),
                                        Yv, Zv, sub)
                bdify(wbd, og)
                yield

                # ---- asinh PS base S4 ----
                cf = ASINH_CF
                pT = psA.tile([128, GW], dt, tag="pT")
                for j in range(GANG):
                    nc.tensor.matmul(sl(pT, j), bd(wbd, j), sl(og, j))   # S2
                nc.scalar.activation(fg[:], pT[:], CopyF)                # S2stk
                bdify(zbd, fg)                                           # S2-BD
                yield
                pT = psA.tile([128, GW], dt, tag="pT")
                for j in range(GANG):
                    nc.tensor.matmul(sl(pT, j), bd(zbd, j), sl(fg, j))   # S4
                # S4-BD only (2 ACT half copies from psum)
                xt, xb = halves(xbd)
                pv = pT[:].rearrange("p (j d) -> p j d", d=N)
                nc.scalar.activation(xt, pv[0:64], CopyF)
                nc.scalar.activation(xb, pv[64:128], CopyF)
                yield

                u = yz[:, 0:GW]
                v = yz[:, GW:2 * GW]
                # leaf: P = cf8 I + cf9 S2 -> v
                nc.vector.scalar_tensor_tensor(v, fg[:], float(cf[9]), cblk(16),
                                               mult, add)
                cur = v
                oth = u
                for i, cbi in ((3, 12), (2, 15), (1, 14), (0, 10)):
                    pQ = psA.tile([128, GW], dt, tag="pT")
                    for j in range(GANG):
                        nc.tensor.matmul(sl(pQ, j), bd(xbd, j), cur[:, j * N:(j + 1) * N])
                    nc.vector.scalar_tensor_tensor(oth, fg[:], float(cf[2 * i + 1]),
                                                   cblk(cbi), mult, add)
                    nc.vector.tensor_tensor(oth, pQ[:], oth, add)
                    cur, oth = oth, cur
                    if i == 2:
                        yield
                pH = psA.tile([128, GW], dt, tag="pT")
                for j in range(GANG):
                    nc.tensor.matmul(sl(pH, j), bd(wbd, j), cur[:, j * N:(j + 1) * N])
                # A = w w^T o (H + GAMMA I) -> fg (overwrites S2stk)
                nc.vector.scalar_tensor_tensor(oth, cI, float(GAMMA), pH[:],
                                               mult, add)
                for j in range(GANG):
                    nc.vector.scalar_tensor_tensor(
                        sl(fg, j), oth[:, j * N:(j + 1) * N], wcolt[:, j:j + 1],
                        sl(wrept, j), mult, mult)
                bdify(zbd, fg)                                           # A-BD
                yield

                # ---- exp PS base A2 ----
                ec = EXP_C
                pT = psA.tile([128, GW], dt, tag="pT")
                for j in range(GANG):
                    nc.tensor.matmul(sl(pT, j), bd(zbd, j), sl(fg, j))   # A2
                nc.scalar.activation(og[:], pT[:], CopyF)                # A2stk
                bdify(xbd, og)                                           # A2-BD
                # P = (ec4 I + ec5 A) + ec6 A2 -> v
                nc.vector.scalar_tensor_tensor(u, fg[:], float(ec[5]), cblk(18),
                                               mult, add)
                nc.vector.scalar_tensor_tensor(v, og[:], float(ec[6]), u, mult, add)
                yield
                pG = psA.tile([128, GW], dt, tag="pT")
                for j in range(GANG):
                    nc.tensor.matmul(sl(pG, j), bd(xbd, j), sl(yz, GANG + j))
                nc.vector.scalar_tensor_tensor(u, fg[:], float(ec[3]), cblk(17),
                                               mult, add)
                nc.vector.tensor_tensor(u, pG[:], u, add)
                pG2 = psA.tile([128, GW], dt, tag="pT")
                for j in range(GANG):
                    nc.tensor.matmul(sl(pG2, j), bd(xbd, j), sl(yz, j))
                nc.vector.scalar_tensor_tensor(v, fg[:], float(ec[1]), cI,
                                               mult, add)
                nc.vector.tensor_tensor(og[:], pG2[:], v, add)           # Gx -> og
                yield

                # ---- 4 squarings (BD in wbd, stacked ping-pong og/gtmp) ----
                chain = [og, gtmp, og, gtmp, og]
                for sq in range(4):
                    src, dst = chain[sq], chain[sq + 1]
                    bdify(wbd, src)
                    pP = psA.tile([128, GW], dt, tag="pT")
                    for j in range(GANG):
                        nc.tensor.matmul(sl(pP, j), bd(wbd, j), sl(src, j))
                    nc.scalar.activation(dst[:], pP[:], CopyF)
                    if sq in (1, 3):
                        yield
                for j in range(GANG):
                    nc.sync.dma_start(o_gv[gi, j], og[:, j * N:(j + 1) * N])

            def run_interleaved(ngang_, width):
                gens = []
                nxt = 0
                while gens or nxt < ngang_:
                    while len(gens) < width and nxt < ngang_:
                        gens.append(_gang_full(nxt))
                        nxt += 1
                    done = []
                    for g in gens:
                        try:
                            next(g)
                        except StopIteration:
                            done.append(g)
                    for g in done:
                        gens.remove(g)

            def _gang_full(gi):
                if BLOCKDIAG:
                    yield from gang_stages_bd(gi)
                else:
                    yield from gang_stages(gi)

            run_interleaved(ngang, INTERLEAVE)

    nc.compile()
    return nc


_cached = {}


def _get_nc(ngang=NGANG):
    if ngang not in _cached:
        _cached[ngang] = build_nc(ngang)
    return _cached[ngang]


def _in_maps(f, weights):
    f32 = np.ascontiguousarray(f[:, 0].astype(np.float32))
    w32 = weights.astype(np.float32)
    cst = _host_constants()
    in_maps = []
    for c in range(N_CORES):
        sl = slice(c * SHARD, (c + 1) * SHARD)
        wcol, wrep = _host_weights(w32[sl])
        in_maps.append({
            "f": np.ascontiguousarray(f32[sl]),
            "wcol": wcol,
            "wrep": wrep,
            "cst": cst,
        })
    return in_maps


def kernel(f: np.ndarray, weights: np.ndarray) -> np.ndarray:
    from concourse.bass_utils import run_bass_kernel_spmd

    assert f.shape == (B_TOTAL, 1, N, N) and weights.shape == (B_TOTAL, N)
    nc = _get_nc()
    res = run_bass_kernel_spmd(nc, _in_maps(f, weights),
                               core_ids=list(range(N_CORES)))
    out = np.empty((B_TOTAL, 1, N, N), np.float32)
    for c in range(N_CORES):
        out[c * SHARD:(c + 1) * SHARD, 0] = res.results[c]["out"]
    return out


def run_traced(f: np.ndarray, weights: np.ndarray):
    from concourse.bass_utils import run_bass_kernel_spmd

    nc = _get_nc()
    return run_bass_kernel_spmd(nc, _in_maps(f, weights),
                                core_ids=list(range(N_CORES)), trace=True)



# revision 3
# speedup vs baseline: 3.6868x; 3.6868x over previous
"""Trainium2 Bass kernel for nn_ADDMeanM_16595753632500.

out[b] = expm(D_b logm(X_b) D_b), X_b = f[b,0] (64x64 SPD), D_b = diag(w[b]),
B = 8192, data-parallel across 8 NeuronCores (1024 samples each).

Eigh-free fp16 algorithm (batched 64x64 matmuls, fp32 PSUM accumulate):
  tuned coupled Newton-Schulz sqrt chain (2 levels, 4+3 iters) ->
      Y ~ c*X^(1/4), Z ~ c'*X^(-1/4)  (recentered; scale folded into
      final-iteration copy scales alpha/beta)
  S = alpha*Y' - beta*Z' = sinh(T), T = (1/4) log x - log r
  Hp = asinh(S)/4 via 4-term odd Horner series
  A = w w^T o (2*Hp + gamma I); out = expm(A)^8 (Taylor-4 Horner +
      3 squarings)
Validated offline vs fp64 eigh oracle: max rel err ~5.2e-3 (gate 2e-2).

Layout: 2 samples per 128 partitions (quadrant K=64 matmuls, base
partitions 0/64), GANG=8 pairs side-by-side in the free dim (512-wide
vector ops amortize 16 samples). fp16 everywhere on-chip except PSUM
(fp32) and const blocks; fp16 DMA in AND out (host casts).
"""
import os
import numpy as np

BUFS_WORK = int(os.environ.get("K_BUFS_WORK", "3"))
BUFS_PSA = int(os.environ.get("K_BUFS_PSA", "4"))
BUFS_PSB = int(os.environ.get("K_BUFS_PSB", "2"))
INTERLEAVE = int(os.environ.get("K_INTERLEAVE", "1"))

# ---------------- tuned schedule constants (offline, /root/tune) ----------
LEV0 = [(1.7545051257294326, -0.23803317376081404),
        (1.5353727795763776, -0.3295560584540806),
        (1.5070719222865991, -0.46490504786416914),
        (1.5002355571599766, -0.49882251009023504)]
LEV1 = [(1.6159520526143833, -0.43106748263419),
        (1.504202789356025, -0.47907982016421485),
        (1.5000816689108767, -0.4995916910213691)]
ALPHA = 0.5284185047966153
BETA = 0.47309797345463184
CC = -0.22121679970910058          # log x = 16*Hp + CC
NSQ = 3
GAMMA = CC / (2 ** NSQ)
ASC = [0.25, -0.25 / 6.0, 0.25 * 3.0 / 40.0, -0.25 * 15.0 / 336.0]

N_CORES = 8
B_TOTAL = 8192
SHARD = B_TOTAL // N_CORES
GANG = 8
N = 64
GW = GANG * N                       # 512
NPAIR = SHARD // 2                  # 512
NGANG = NPAIR // GANG               # 64

# const blocks (each GW wide), fp32:
#  0..3 a of LEV0, 4..6 a of LEV1, 7 cf2, 8 cf1, 9 cf0, 10 gamma, 11 one
_CONST_VALS = ([a for (a, b) in LEV0] + [a for (a, b) in LEV1]
               + [ASC[2], ASC[1], ASC[0], GAMMA, 1.0])
NCONST = len(_CONST_VALS)


def _host_constants():
    eye = np.eye(N, dtype=np.float32)
    blk = np.zeros((128, NCONST * GW), np.float32)
    for k, v in enumerate(_CONST_VALS):
        for j in range(GANG):
            for t in range(2):
                blk[64 * t:64 * t + 64, k * GW + j * N:k * GW + (j + 1) * N] \
                    = v * eye
    return blk


def _rearr(x):
    """[SHARD, 64, 64] -> [NGANG, 128, GW] gang layout (sample s=(g*8+j)*2+t
    lives at partitions 64t..64t+64, cols 64j..64j+64)."""
    v = x.reshape(NGANG, GANG, 2, N, N).transpose(0, 2, 3, 1, 4)
    return np.ascontiguousarray(v.reshape(NGANG, 128, GW))


def _unrearr(y):
    """inverse of _rearr."""
    v = y.reshape(NGANG, 2, N, GANG, N).transpose(0, 3, 1, 2, 4)
    return v.reshape(SHARD, N, N)


def build_nc(ngang=NGANG):
    import concourse.bacc as bacc
    import concourse.mybir as mybir
    import concourse.tile as tile

    f32 = mybir.dt.float32
    f16 = mybir.dt.float16
    nc = bacc.Bacc()
    f_in = nc.declare_dram_parameter("f", [ngang, 128, GW], f16,
                                     isOutput=False)
    wo_in = nc.declare_dram_parameter("wo", [ngang, 128, GW], f16,
                                      isOutput=False)
    cst_in = nc.declare_dram_parameter("cst", [128, NCONST * GW], f32,
                                       isOutput=False)
    out_d = nc.declare_dram_parameter("out", [ngang, 128, GW], f16,
                                      isOutput=True)

    mult = mybir.AluOpType.mult
    add = mybir.AluOpType.add
    sub = mybir.AluOpType.subtract
    CopyF = mybir.ActivationFunctionType.Copy

    with tile.TileContext(nc) as tc:
        with (
            tc.tile_pool(name="consts", bufs=1) as cpool,
            tc.tile_pool(name="work", bufs=BUFS_WORK) as wpool,
            tc.tile_pool(name="psA", bufs=BUFS_PSA, space="PSUM") as psA,
            tc.tile_pool(name="psB", bufs=BUFS_PSB, space="PSUM") as psB,
        ):
            cst = cpool.tile([128, NCONST * GW], f32)
            nc.sync.dma_start(cst[:], cst_in[:])

            def cblk(k):
                return cst[:, k * GW:(k + 1) * GW]
            cI = cblk(11)

            def gang_stages(gi):
                xg = wpool.tile([128, GW], f16, tag="xg")
                wog = wpool.tile([128, GW], f16, tag="wog")
                yz = wpool.tile([128, 2 * GW], f16, tag="yz")
                yz2 = wpool.tile([128, 2 * GW], f16, tag="yz2")
                wt = wpool.tile([128, GW], f16, tag="wt")
                sg = wpool.tile([128, GW], f16, tag="sg")
                ug = wpool.tile([128, GW], f16, tag="ug")
                pg = wpool.tile([128, GW], f16, tag="pg")
                pg2 = wpool.tile([128, GW], f16, tag="pg2")
                t32 = wpool.tile([128, GW], f32, tag="t32")
                ag = wpool.tile([128, GW], f16, tag="ag")
                gg = wpool.tile([128, GW], f16, tag="gg")
                gg2 = wpool.tile([128, GW], f16, tag="gg2")
                og = wpool.tile([128, GW], f16, tag="og")

                nc.sync.dma_start(xg[:], f_in[gi])
                nc.sync.dma_start(wog[:], wo_in[gi])
                yield

                def sl(tile_, j):
                    return tile_[:, j * N:(j + 1) * N]

                def mmq(out_ap, statT_ap, mov_ap):
                    for t in range(2):
                        ps = slice(64 * t, 64 * t + 64)
                        nc.tensor.matmul(out_ap[ps], statT_ap[ps], mov_ap[ps])

                def prod(pool, statT, mov, wide=False, tag="pT"):
                    w = 2 * GW if wide else GW
                    p = pool.tile([128, w], f32, tag=tag)
                    for j in range(GANG):
                        if wide:
                            mmq(p[:, 2 * N * j:2 * N * (j + 1)],
                                sl(statT, j), mov(j))
                        else:
                            mmq(sl(p, j), sl(statT, j), mov(j))
                    return p

                def yzv(tile_):
                    return tile_[:].rearrange("p (j d) -> p j d", d=2 * N)

                def Ysl(tile_, j):
                    return tile_[:, 2 * N * j: 2 * N * j + N]

                def Zsl(tile_, j):
                    return tile_[:, 2 * N * j + N: 2 * N * j + 2 * N]

                def YZsl(tile_, j):
                    return tile_[:, 2 * N * j: 2 * N * (j + 1)]

                # ---- level 0 ----
                # i1: W = a0 I + b0 X; Y1 = W@X; Z1 = W
                nc.vector.scalar_tensor_tensor(
                    wt[:], xg[:], float(LEV0[0][1]), cblk(0), mult, add)
                pY = prod(psA, wt, lambda j: sl(xg, j))
                Yv = yzv(yz)[:, :, 0:N]
                Zv = yzv(yz)[:, :, N:2 * N]
                nc.scalar.activation(Yv, pY[:].rearrange("p (j d) -> p j d",
                                                         d=N), CopyF)
                nc.gpsimd.tensor_copy(Zv, wt[:].rearrange("p (j d) -> p j d",
                                                          d=N))
                yield

                def ns_iter(src, dst, k, lev, cb, last=False):
                    # T = Z@Y ; W = a I + b T ; [Y'|Z'] = W @ [Y|Z]
                    b = lev[k][1]
                    pT = prod(psA, None, None, tag="pT") if False else None
                    pT = psA.tile([128, GW], f32, tag="pT")
                    for j in range(GANG):
                        mmq(sl(pT, j), Zsl(src, j), Ysl(src, j))
                    nc.vector.scalar_tensor_tensor(
                        wt[:], pT[:], float(b), cblk(cb), mult, add)
                    pYZ = psB.tile([128, 2 * GW], f32, tag="pYZ")
                    for j in range(GANG):
                        mmq(pYZ[:, 2 * N * j:2 * N * (j + 1)], sl(wt, j),
                            YZsl(src, j))
                    pv = pYZ[:].rearrange("p (j d) -> p j d", d=2 * N)
                    if last:
                        # zb = beta*Z' ; S = alpha*Y' - zb
                        nc.scalar.activation(ug[:].rearrange(
                            "p (j d) -> p j d", d=N), pv[:, :, N:2 * N],
                            CopyF, scale=float(BETA))
                        nc.vector.scalar_tensor_tensor(
                            sg[:].rearrange("p (j d) -> p j d", d=N),
                            pv[:, :, 0:N], float(ALPHA),
                            ug[:].rearrange("p (j d) -> p j d", d=N),
                            mult, sub)
                    else:
                        nc.scalar.activation(dst[:], pYZ[:], CopyF)

                for k in range(1, len(LEV0)):
                    ns_iter(yz, yz, k, LEV0, k)
                    yield

                # ---- level 1 ----
                # i1: W = a I + b Y ; Ynew = W@Y ; Znew = W
                Y2v = yzv(yz2)[:, :, 0:N]
                Z2v = yzv(yz2)[:, :, N:2 * N]
                nc.vector.scalar_tensor_tensor(
                    wt[:].rearrange("p (j d) -> p j d", d=N), Yv,
                    float(LEV1[0][1]),
                    cblk(4)[:].rearrange("p (j d) -> p j d", d=N), mult, add)
                pY2 = psA.tile([128, GW], f32, tag="pT")
                for j in range(GANG):
                    mmq(sl(pY2, j), sl(wt, j), Ysl(yz, j))
                nc.scalar.activation(Y2v, pY2[:].rearrange(
                    "p (j d) -> p j d", d=N), CopyF)
                nc.gpsimd.tensor_copy(Z2v, wt[:].rearrange(
                    "p (j d) -> p j d", d=N))
                yield

                ns_iter(yz2, yz2, 1, LEV1, 5)
                yield
                ns_iter(yz2, yz2, 2, LEV1, 6, last=True)
                yield

                # ---- asinh: Hp = S * (cf0 I + cf1 U + cf2 U^2 + cf3 U^3),
                #      U = S^2, Horner ----
                pU = psA.tile([128, GW], f32, tag="pT")
                for j in range(GANG):
                    mmq(sl(pU, j), sl(sg, j), sl(sg, j))
                nc.scalar.activation(ug[:], pU[:], CopyF)
                yield
                # P2 = cf3 U + cf2 I
                nc.vector.scalar_tensor_tensor(
                    pg[:], ug[:], float(ASC[3]), cblk(7), mult, add)
                # P1 = P2@U + cf1 I
                pP = psA.tile([128, GW], f32, tag="pT")
                for j in range(GANG):
                    mmq(sl(pP, j), sl(ug, j), sl(pg, j))
                nc.vector.scalar_tensor_tensor(
                    pg2[:], pP[:], 1.0, cblk(8), mult, add)
                # P0 = P1@U + cf0 I
                pP0 = psA.tile([128, GW], f32, tag="pT")
                for j in range(GANG):
                    mmq(sl(pP0, j), sl(ug, j), sl(pg2, j))
                nc.vector.scalar_tensor_tensor(
                    pg[:], pP0[:], 1.0, cblk(9), mult, add)
                yield
                # Hp = S @ P0 ; A = wo o (2 Hp + gamma I)
                pH = psA.tile([128, GW], f32, tag="pT")
                for j in range(GANG):
                    mmq(sl(pH, j), sl(sg, j), sl(pg, j))
                nc.vector.scalar_tensor_tensor(
                    t32[:], pH[:], float(16.0 / (2 ** NSQ)), cblk(10),
                    mult, add)
                nc.gpsimd.tensor_tensor(ag[:], t32[:], wog[:], mult)
                yield

                # ---- exp Taylor-4 Horner: G = I + A(I + A/2(I + A/3(I+A/4)))
                nc.vector.scalar_tensor_tensor(
                    gg[:], ag[:], 0.25, cI, mult, add)
                pG = psA.tile([128, GW], f32, tag="pT")
                for j in range(GANG):
                    mmq(sl(pG, j), sl(ag, j), sl(gg, j))
                nc.vector.scalar_tensor_tensor(
                    gg2[:], pG[:], float(1.0 / 3.0), cI, mult, add)
                pG2 = psA.tile([128, GW], f32, tag="pT")
                for j in range(GANG):
                    mmq(sl(pG2, j), sl(ag, j), sl(gg2, j))
                nc.vector.scalar_tensor_tensor(
                    gg[:], pG2[:], 0.5, cI, mult, add)
                yield
                pG3 = psA.tile([128, GW], f32, tag="pT")
                for j in range(GANG):
                    mmq(sl(pG3, j), sl(ag, j), sl(gg, j))
                nc.vector.scalar_tensor_tensor(
                    gg2[:], pG3[:], 1.0, cI, mult, add)

                # ---- 3 squarings ----
                pS1 = psA.tile([128, GW], f32, tag="pT")
                for j in range(GANG):
                    mmq(sl(pS1, j), sl(gg2, j), sl(gg2, j))
                nc.scalar.activation(gg[:], pS1[:], CopyF)
                yield
                pS2 = psA.tile([128, GW], f32, tag="pT")
                for j in range(GANG):
                    mmq(sl(pS2, j), sl(gg, j), sl(gg, j))
                nc.scalar.activation(gg2[:], pS2[:], CopyF)
                pS3 = psA.tile([128, GW], f32, tag="pT")
                for j in range(GANG):
                    mmq(sl(pS3, j), sl(gg2, j), sl(gg2, j))
                nc.scalar.activation(og[:], pS3[:], CopyF)
                nc.sync.dma_start(out_d[gi], og[:])

            def run_interleaved(ngang_, width):
                gens = []
                nxt = 0
                while gens or nxt < ngang_:
                    while len(gens) < width and nxt < ngang_:
                        gens.append(gang_stages(nxt))
                        nxt += 1
                    done = []
                    for g in gens:
                        try:
                            next(g)
                        except StopIteration:
                            done.append(g)
                    for g in done:
                        gens.remove(g)

            run_interleaved(ngang, INTERLEAVE)

    nc.compile()
    return nc


_cached = {}


def _get_nc(ngang=NGANG):
    if ngang not in _cached:
        _cached[ngang] = build_nc(ngang)
    return _cached[ngang]


def _in_maps(f, weights):
    f16 = f[:, 0].astype(np.float16)
    w32 = weights.astype(np.float32)
    cst = _host_constants()
    in_maps = []
    for c in range(N_CORES):
        sl_ = slice(c * SHARD, (c + 1) * SHARD)
        wc = w32[sl_]
        wo = (wc[:, :, None] * wc[:, None, :]).astype(np.float16)
        in_maps.append({
            "f": _rearr(f16[sl_]),
            "wo": _rearr(wo),
            "cst": cst,
        })
    return in_maps


def kernel(f: np.ndarray, weights: np.ndarray) -> np.ndarray:
    from concourse.bass_utils import run_bass_kernel_spmd

    assert f.shape == (B_TOTAL, 1, N, N) and weights.shape == (B_TOTAL, N)
    nc = _get_nc()
    res = run_bass_kernel_spmd(nc, _in_maps(f, weights),
                               core_ids=list(range(N_CORES)))
    out = np.empty((B_TOTAL, 1, N, N), np.float32)
    for c in range(N_CORES):
        out[c * SHARD:(c + 1) * SHARD, 0] = \
            _unrearr(res.results[c]["out"]).astype(np.float32)
    return out


def run_traced(f: np.ndarray, weights: np.ndarray):
    from concourse.bass_utils import run_bass_kernel_spmd

    nc = _get_nc()
    return run_bass_kernel_spmd(nc, _in_maps(f, weights),
                                core_ids=list(range(N_CORES)), trace=True)


# revision 6
# speedup vs baseline: 4.7526x; 1.2891x over previous
"""Trainium2 Bass kernel for nn_ADDMeanM_16595753632500.

out[b] = expm(D_b logm(X_b) D_b), X_b = f[b,0] (64x64 SPD), D_b = diag(w[b]),
B = 8192, data-parallel across 8 NeuronCores (1024 samples each).

Eigh-free fp16 algorithm (batched 64x64 matmuls, fp32 PSUM accumulate):
  tuned coupled Newton-Schulz sqrt chain (2 levels, 4+3 iters) ->
      Y ~ c*X^(1/4), Z ~ c'*X^(-1/4)  (recentered; scale folded into
      final-iteration copy scales alpha/beta)
  S = alpha*Y' - beta*Z' = sinh(T), T = (1/4) log x - log r
  Hp = asinh(S)/4 via 4-term odd Horner series
  A = w w^T o (2*Hp + gamma I); out = expm(A)^8 (Taylor-4 Horner +
      3 squarings)
Validated offline vs fp64 eigh oracle: max rel err ~5.2e-3 (gate 2e-2).

Layout: 2 samples per 128 partitions (quadrant K=64 matmuls, base
partitions 0/64), GANG=8 pairs side-by-side in the free dim (512-wide
vector ops amortize 16 samples). fp16 everywhere on-chip except PSUM
(fp32) and const blocks; fp16 DMA in AND out (host casts).
"""
import os
import numpy as np

BUFS_WORK = int(os.environ.get("K_BUFS_WORK", "3"))
BUFS_PSA = int(os.environ.get("K_BUFS_PSA", "4"))
BUFS_PSB = int(os.environ.get("K_BUFS_PSB", "2"))
INTERLEAVE = int(os.environ.get("K_INTERLEAVE", "8"))

# ---------------- tuned schedule constants (offline, /root/tune) ----------
LEV0 = [(1.7545051257294326, -0.23803317376081404),
        (1.5353727795763776, -0.3295560584540806),
        (1.5070719222865991, -0.46490504786416914),
        (1.5002355571599766, -0.49882251009023504)]
LEV1 = [(1.6159520526143833, -0.43106748263419),
        (1.504202789356025, -0.47907982016421485),
        (1.5000816689108767, -0.4995916910213691)]
ALPHA = 0.5284185047966153
BETA = 0.47309797345463184
CC = -0.22121679970910058          # log x = 16*Hp + CC
NSQ = 3
GAMMA = CC / (2 ** NSQ)
ASC = [0.25, -0.25 / 6.0, 0.25 * 3.0 / 40.0, -0.25 * 15.0 / 336.0]

N_CORES = 8
B_TOTAL = 8192
SHARD = B_TOTAL // N_CORES
GANG = 8
N = 64
GW = GANG * N                       # 512
NPAIR = SHARD // 2                  # 512
NGANG = NPAIR // GANG               # 64

# const blocks (each GW wide), fp32:
#  0..3 a of LEV0, 4..6 a of LEV1, 7 cf2, 8 cf1, 9 cf0, 10 gamma, 11 one
_CONST_VALS = ([a for (a, b) in LEV0] + [a for (a, b) in LEV1]
               + [ASC[2], ASC[1], ASC[0], GAMMA, 1.0])
NCONST = len(_CONST_VALS)


def _host_constants():
    eye = np.eye(N, dtype=np.float32)
    blk = np.zeros((128, NCONST * GW), np.float32)
    for k, v in enumerate(_CONST_VALS):
        for j in range(GANG):
            for t in range(2):
                blk[64 * t:64 * t + 64, k * GW + j * N:k * GW + (j + 1) * N] \
                    = v * eye
    return blk


def _rearr(x):
    """[SHARD, 64, 64] -> [NGANG, 128, GW] gang layout (sample s=(g*8+j)*2+t
    lives at partitions 64t..64t+64, cols 64j..64j+64)."""
    v = x.reshape(NGANG, GANG, 2, N, N).transpose(0, 2, 3, 1, 4)
    return np.ascontiguousarray(v.reshape(NGANG, 128, GW))


def _unrearr(y):
    """inverse of _rearr."""
    v = y.reshape(NGANG, 2, N, GANG, N).transpose(0, 3, 1, 2, 4)
    return v.reshape(SHARD, N, N)


def build_nc(ngang=NGANG):
    import concourse.bacc as bacc
    import concourse.mybir as mybir
    import concourse.tile as tile

    f32 = mybir.dt.float32
    f16 = mybir.dt.float16
    nc = bacc.Bacc()
    f_in = nc.declare_dram_parameter("f", [ngang, 128, GW], f16,
                                     isOutput=False)
    wo_in = nc.declare_dram_parameter("wo", [ngang, 128, GW], f16,
                                      isOutput=False)
    cst_in = nc.declare_dram_parameter("cst", [128, NCONST * GW], f32,
                                       isOutput=False)
    out_d = nc.declare_dram_parameter("out", [ngang, 128, GW], f16,
                                      isOutput=True)

    mult = mybir.AluOpType.mult
    add = mybir.AluOpType.add
    sub = mybir.AluOpType.subtract
    CopyF = mybir.ActivationFunctionType.Copy

    with tile.TileContext(nc) as tc:
        with (
            tc.tile_pool(name="consts", bufs=1) as cpool,
            tc.tile_pool(name="work", bufs=BUFS_WORK) as wpool,
            tc.tile_pool(name="psA", bufs=BUFS_PSA, space="PSUM") as psA,
            tc.tile_pool(name="psB", bufs=BUFS_PSB, space="PSUM") as psB,
        ):
            cst = cpool.tile([128, NCONST * GW], f32)
            nc.sync.dma_start(cst[:], cst_in[:])

            def cblk(k):
                return cst[:, k * GW:(k + 1) * GW]
            cI = cblk(11)

            def gang_stages(gi):
                xg = wpool.tile([128, GW], f16, tag="xg")
                wog = wpool.tile([128, GW], f16, tag="wog")
                yz = wpool.tile([128, 2 * GW], f16, tag="yz")
                yz2 = wpool.tile([128, 2 * GW], f16, tag="yz2")
                wt = wpool.tile([128, GW], f16, tag="wt")
                sg = wpool.tile([128, GW], f16, tag="sg")
                ug = wpool.tile([128, GW], f16, tag="ug")
                pg = wpool.tile([128, GW], f16, tag="pg")
                pg2 = wpool.tile([128, GW], f16, tag="pg2")
                t32 = wpool.tile([128, GW], f32, tag="t32")
                ag = xg                      # X dead after L0 i1
                gg = yz[:, 0:GW]             # yz dead after L1 i1
                gg2 = yz[:, GW:2 * GW]
                og = ug                      # U dead after Hp

                nc.sync.dma_start(xg[:], f_in[gi])
                nc.sync.dma_start(wog[:], wo_in[gi])
                yield

                def sl(tile_, j):
                    return tile_[:, j * N:(j + 1) * N]

                def mmq(out_ap, statT_ap, mov_ap):
                    for t in range(2):
                        ps = slice(64 * t, 64 * t + 64)
                        nc.tensor.matmul(out_ap[ps], statT_ap[ps], mov_ap[ps])

                def prod(pool, statT, mov, wide=False, tag="pT"):
                    w = 2 * GW if wide else GW
                    p = pool.tile([128, w], f32, tag=tag)
                    for j in range(GANG):
                        if wide:
                            mmq(p[:, 2 * N * j:2 * N * (j + 1)],
                                sl(statT, j), mov(j))
                        else:
                            mmq(sl(p, j), sl(statT, j), mov(j))
                    return p

                def yzv(tile_):
                    return tile_[:].rearrange("p (j d) -> p j d", d=2 * N)

                def Ysl(tile_, j):
                    return tile_[:, 2 * N * j: 2 * N * j + N]

                def Zsl(tile_, j):
                    return tile_[:, 2 * N * j + N: 2 * N * j + 2 * N]

                def YZsl(tile_, j):
                    return tile_[:, 2 * N * j: 2 * N * (j + 1)]

                # ---- level 0 ----
                # i1: W = a0 I + b0 X; Y1 = W@X; Z1 = W
                nc.vector.scalar_tensor_tensor(
                    wt[:], xg[:], float(LEV0[0][1]), cblk(0), mult, add)
                pY = prod(psA, wt, lambda j: sl(xg, j))
                Yv = yzv(yz)[:, :, 0:N]
                Zv = yzv(yz)[:, :, N:2 * N]
                nc.scalar.activation(Yv, pY[:].rearrange("p (j d) -> p j d",
                                                         d=N), CopyF)
                nc.gpsimd.tensor_copy(Zv, wt[:].rearrange("p (j d) -> p j d",
                                                          d=N))
                yield

                def ns_iter(src, dst, k, lev, cb, last=False):
                    # T = Z@Y ; W = a I + b T ; [Y'|Z'] = W @ [Y|Z]
                    b = lev[k][1]
                    pT = psA.tile([128, GW], f32, tag="pT")
                    for j in range(GANG):
                        mmq(sl(pT, j), Zsl(src, j), Ysl(src, j))
                    nc.vector.scalar_tensor_tensor(
                        wt[:], pT[:], float(b), cblk(cb), mult, add)
                    pYZ = psB.tile([128, 2 * GW], f32, tag="pYZ")
                    for j in range(GANG):
                        mmq(pYZ[:, 2 * N * j:2 * N * (j + 1)], sl(wt, j),
                            YZsl(src, j))
                    pv = pYZ[:].rearrange("p (j d) -> p j d", d=2 * N)
                    if last:
                        # zb = beta*Z' ; S = alpha*Y' - zb
                        nc.scalar.activation(ug[:].rearrange(
                            "p (j d) -> p j d", d=N), pv[:, :, N:2 * N],
                            CopyF, scale=float(BETA))
                        nc.vector.scalar_tensor_tensor(
                            sg[:].rearrange("p (j d) -> p j d", d=N),
                            pv[:, :, 0:N], float(ALPHA),
                            ug[:].rearrange("p (j d) -> p j d", d=N),
                            mult, sub)
                    else:
                        nc.scalar.activation(dst[:], pYZ[:], CopyF)

                for k in range(1, len(LEV0)):
                    ns_iter(yz, yz, k, LEV0, k)
                    yield

                # ---- level 1 ----
                # i1: W = a I + b Y ; Ynew = W@Y ; Znew = W
                Y2v = yzv(yz2)[:, :, 0:N]
                Z2v = yzv(yz2)[:, :, N:2 * N]
                nc.vector.scalar_tensor_tensor(
                    wt[:].rearrange("p (j d) -> p j d", d=N), Yv,
                    float(LEV1[0][1]),
                    cblk(4)[:].rearrange("p (j d) -> p j d", d=N), mult, add)
                pY2 = psA.tile([128, GW], f32, tag="pT")
                for j in range(GANG):
                    mmq(sl(pY2, j), sl(wt, j), Ysl(yz, j))
                nc.scalar.activation(Y2v, pY2[:].rearrange(
                    "p (j d) -> p j d", d=N), CopyF)
                nc.gpsimd.tensor_copy(Z2v, wt[:].rearrange(
                    "p (j d) -> p j d", d=N))
                yield

                ns_iter(yz2, yz2, 1, LEV1, 5)
                yield
                ns_iter(yz2, yz2, 2, LEV1, 6, last=True)
                yield

                # ---- asinh Horner: Hp = S*(cf0 I + cf1 U + cf2 U^2
                #      + cf3 U^3), U = S^2 ----
                pU = psA.tile([128, GW], f32, tag="pT")
                for j in range(GANG):
                    mmq(sl(pU, j), sl(sg, j), sl(sg, j))
                nc.scalar.activation(ug[:], pU[:], CopyF)
                yield
                nc.vector.scalar_tensor_tensor(
                    pg[:], ug[:], float(ASC[3]), cblk(7), mult, add)
                pP = psA.tile([128, GW], f32, tag="pT")
                for j in range(GANG):
                    mmq(sl(pP, j), sl(ug, j), sl(pg, j))
                nc.vector.scalar_tensor_tensor(
                    pg2[:], pP[:], 1.0, cblk(8), mult, add)
                pP0 = psA.tile([128, GW], f32, tag="pT")
                for j in range(GANG):
                    mmq(sl(pP0, j), sl(ug, j), sl(pg2, j))
                nc.vector.scalar_tensor_tensor(
                    pg[:], pP0[:], 1.0, cblk(9), mult, add)
                yield
                # Hp = S @ P0 ; A = wo o (2 Hp + gamma I)
                pH = psA.tile([128, GW], f32, tag="pT")
                for j in range(GANG):
                    mmq(sl(pH, j), sl(sg, j), sl(pg, j))
                nc.vector.scalar_tensor_tensor(
                    t32[:], pH[:], float(16.0 / (2 ** NSQ)), cblk(10),
                    mult, add)
                nc.gpsimd.tensor_tensor(ag[:], t32[:], wog[:], mult)
                yield

                # ---- exp Taylor-4 Horner: G = I + A(I + A/2(I + A/3(I+A/4)))
                nc.vector.scalar_tensor_tensor(
                    gg, ag[:], 0.25, cI, mult, add)
                pG = psA.tile([128, GW], f32, tag="pT")
                for j in range(GANG):
                    mmq(sl(pG, j), sl(ag, j), gg[:, j * N:(j + 1) * N])
                nc.vector.scalar_tensor_tensor(
                    gg2, pG[:], float(1.0 / 3.0), cI, mult, add)
                pG2 = psA.tile([128, GW], f32, tag="pT")
                for j in range(GANG):
                    mmq(sl(pG2, j), sl(ag, j), gg2[:, j * N:(j + 1) * N])
                nc.vector.scalar_tensor_tensor(
                    gg, pG2[:], 0.5, cI, mult, add)
                yield
                pG3 = psA.tile([128, GW], f32, tag="pT")
                for j in range(GANG):
                    mmq(sl(pG3, j), sl(ag, j), gg[:, j * N:(j + 1) * N])
                nc.vector.scalar_tensor_tensor(
                    gg2, pG3[:], 1.0, cI, mult, add)

                # ---- 3 squarings ----
                pS1 = psA.tile([128, GW], f32, tag="pT")
                for j in range(GANG):
                    mmq(sl(pS1, j), gg2[:, j * N:(j + 1) * N], gg2[:, j * N:(j + 1) * N])
                nc.scalar.activation(gg, pS1[:], CopyF)
                yield
                pS2 = psA.tile([128, GW], f32, tag="pT")
                for j in range(GANG):
                    mmq(sl(pS2, j), gg[:, j * N:(j + 1) * N], gg[:, j * N:(j + 1) * N])
                nc.scalar.activation(gg2, pS2[:], CopyF)
                pS3 = psA.tile([128, GW], f32, tag="pT")
                for j in range(GANG):
                    mmq(sl(pS3, j), gg2[:, j * N:(j + 1) * N], gg2[:, j * N:(j + 1) * N])
                nc.scalar.activation(og[:], pS3[:], CopyF)
                nc.sync.dma_start(out_d[gi], og[:])

            def run_interleaved(ngang_, width):
                gens = []
                nxt = 0
                while gens or nxt < ngang_:
                    if len(gens) < width and nxt < ngang_:
                        gens.append(gang_stages(nxt))
                        nxt += 1
                    done = []
                    for g in gens:
                        try:
                            next(g)
                        except StopIteration:
                            done.append(g)
                    for g in done:
                        gens.remove(g)

            run_interleaved(ngang, INTERLEAVE)

    nc.compile()
    return nc


_cached = {}


def _get_nc(ngang=NGANG):
    if ngang not in _cached:
        _cached[ngang] = build_nc(ngang)
    return _cached[ngang]


def _in_maps(f, weights):
    f16 = f[:, 0].astype(np.float16)
    w32 = weights.astype(np.float32)
    cst = _host_constants()
    in_maps = []
    for c in range(N_CORES):
        sl_ = slice(c * SHARD, (c + 1) * SHARD)
        wc = w32[sl_]
        wo = (wc[:, :, None] * wc[:, None, :]).astype(np.float16)
        in_maps.append({
            "f": _rearr(f16[sl_]),
            "wo": _rearr(wo),
            "cst": cst,
        })
    return in_maps


def kernel(f: np.ndarray, weights: np.ndarray) -> np.ndarray:
    from concourse.bass_utils import run_bass_kernel_spmd

    assert f.shape == (B_TOTAL, 1, N, N) and weights.shape == (B_TOTAL, N)
    nc = _get_nc()
    res = run_bass_kernel_spmd(nc, _in_maps(f, weights),
                               core_ids=list(range(N_CORES)))
    out = np.empty((B_TOTAL, 1, N, N), np.float32)
    for c in range(N_CORES):
        out[c * SHARD:(c + 1) * SHARD, 0] = \
            _unrearr(res.results[c]["out"]).astype(np.float32)
    return out


def run_traced(f: np.ndarray, weights: np.ndarray):
    from concourse.bass_utils import run_bass_kernel_spmd

    nc = _get_nc()
    return run_bass_kernel_spmd(nc, _in_maps(f, weights),
                                core_ids=list(range(N_CORES)), trace=True)


# revision 7
# speedup vs baseline: 6.0774x; 1.2788x over previous
"""Trainium2 Bass kernel for nn_ADDMeanM_16595753632500.

out[b] = expm(D_b logm(X_b) D_b), X_b = f[b,0] (64x64 SPD), D_b = diag(w[b]),
B = 8192, data-parallel across 8 NeuronCores (1024 samples each).

Eigh-free fp16 algorithm (batched 64x64 matmuls, fp32 PSUM accumulate):
  tuned coupled Newton-Schulz sqrt chain (2 levels, 4+3 iters) ->
      Y ~ c*X^(1/4), Z ~ c'*X^(-1/4)  (recentered; scale folded into
      final-iteration copy scales alpha/beta)
  S = alpha*Y' - beta*Z' = sinh(T), T = (1/4) log x - log r
  Hp = asinh(S)/4 via 4-term odd Horner series
  A = w w^T o (2*Hp + gamma I); out = expm(A)^8 (Taylor-4 Horner +
      3 squarings)
Validated offline vs fp64 eigh oracle: max rel err ~5.2e-3 (gate 2e-2).

Layout: 2 samples per 128 partitions (quadrant K=64 matmuls, base
partitions 0/64), GANG=8 pairs side-by-side in the free dim (512-wide
vector ops amortize 16 samples). fp16 everywhere on-chip except PSUM
(fp32) and const blocks; fp16 DMA in AND out (host casts).
"""
import os
import numpy as np

BUFS_WORK = int(os.environ.get("K_BUFS_WORK", "3"))
BUFS_PSA = int(os.environ.get("K_BUFS_PSA", "4"))
BUFS_PSB = int(os.environ.get("K_BUFS_PSB", "2"))
INTERLEAVE = int(os.environ.get("K_INTERLEAVE", "8"))

# ---------------- tuned schedule constants (offline, /root/tune) ----------
LEV0 = [(1.7545051257294326, -0.23803317376081404),
        (1.5353727795763776, -0.3295560584540806),
        (1.5070719222865991, -0.46490504786416914),
        (1.5002355571599766, -0.49882251009023504)]
LEV1 = [(1.6159520526143833, -0.43106748263419),
        (1.504202789356025, -0.47907982016421485),
        (1.5000816689108767, -0.4995916910213691)]
ALPHA = 0.5284185047966153
BETA = 0.47309797345463184
CC = -0.22121679970910058          # log x = 16*Hp + CC
NSQ = 2
GAMMA = CC / (2 ** NSQ)
ASC = [0.25, -0.25 / 6.0, 0.25 * 3.0 / 40.0, -0.25 * 15.0 / 336.0]

N_CORES = 8
B_TOTAL = 8192
SHARD = B_TOTAL // N_CORES
GANG = 8
N = 64
GW = GANG * N                       # 512
NPAIR = SHARD // 2                  # 512
NGANG = NPAIR // GANG               # 64

# const blocks (each GW wide), fp32:
#  0..3 a of LEV0, 4..6 a of LEV1, 7 cf0, 8 gamma, 9 one
_CONST_VALS = ([a for (a, b) in LEV0] + [a for (a, b) in LEV1]
               + [ASC[0], GAMMA / 4.0, 1.0])
NCONST = len(_CONST_VALS)
# fp16 const blocks (for DVE tensor_tensor adds): 0 one, 1 cf1
_CONST16_VALS = [1.0, ASC[1]]
NCONST16 = len(_CONST16_VALS)


def _host_constants():
    eye = np.eye(N, dtype=np.float32)
    blk = np.zeros((128, NCONST * GW), np.float32)
    for k, v in enumerate(_CONST_VALS):
        for j in range(GANG):
            for t in range(2):
                blk[64 * t:64 * t + 64, k * GW + j * N:k * GW + (j + 1) * N] \
                    = v * eye
    return blk


def _host_constants16():
    eye = np.eye(N, dtype=np.float16)
    blk = np.zeros((128, NCONST16 * GW), np.float16)
    for k, v in enumerate(_CONST16_VALS):
        for j in range(GANG):
            for t in range(2):
                blk[64 * t:64 * t + 64, k * GW + j * N:k * GW + (j + 1) * N] \
                    = np.float16(v) * eye
    return blk


def _rearr(x):
    """[SHARD, 64, 64] -> [NGANG, 128, GW] gang layout (sample s=(g*8+j)*2+t
    lives at partitions 64t..64t+64, cols 64j..64j+64)."""
    v = x.reshape(NGANG, GANG, 2, N, N).transpose(0, 2, 3, 1, 4)
    return np.ascontiguousarray(v.reshape(NGANG, 128, GW))


def _unrearr(y):
    """inverse of _rearr."""
    v = y.reshape(NGANG, 2, N, GANG, N).transpose(0, 3, 1, 2, 4)
    return v.reshape(SHARD, N, N)


def build_nc(ngang=NGANG):
    import concourse.bacc as bacc
    import concourse.mybir as mybir
    import concourse.tile as tile

    f32 = mybir.dt.float32
    f16 = mybir.dt.float16
    nc = bacc.Bacc()
    f_in = nc.declare_dram_parameter("f", [ngang, 128, GW], f16,
                                     isOutput=False)
    wo_in = nc.declare_dram_parameter("wo", [ngang, 128, GW], f16,
                                      isOutput=False)
    cst_in = nc.declare_dram_parameter("cst", [128, NCONST * GW], f32,
                                       isOutput=False)
    cst16_in = nc.declare_dram_parameter("cst16", [128, NCONST16 * GW], f16,
                                         isOutput=False)
    out_d = nc.declare_dram_parameter("out", [ngang, 128, GW], f16,
                                      isOutput=True)

    mult = mybir.AluOpType.mult
    add = mybir.AluOpType.add
    sub = mybir.AluOpType.subtract
    CopyF = mybir.ActivationFunctionType.Copy

    with tile.TileContext(nc) as tc:
        with (
            tc.tile_pool(name="consts", bufs=1) as cpool,
            tc.tile_pool(name="work", bufs=BUFS_WORK) as wpool,
            tc.tile_pool(name="psA", bufs=BUFS_PSA, space="PSUM") as psA,
            tc.tile_pool(name="psB", bufs=BUFS_PSB, space="PSUM") as psB,
        ):
            cst = cpool.tile([128, NCONST * GW], f32)
            nc.sync.dma_start(cst[:], cst_in[:])
            cst16 = cpool.tile([128, NCONST16 * GW], f16)
            nc.sync.dma_start(cst16[:], cst16_in[:])

            def cblk(k):
                return cst[:, k * GW:(k + 1) * GW]

            def cblk16(k):
                return cst16[:, k * GW:(k + 1) * GW]
            cI = cblk(9)
            cI16 = cblk16(0)

            def gang_stages(gi):
                xg = wpool.tile([128, GW], f16, tag="xg")
                wog = wpool.tile([128, GW], f16, tag="wog")
                yz = wpool.tile([128, 2 * GW], f16, tag="yz")
                yz2 = wpool.tile([128, 2 * GW], f16, tag="yz2")
                wt = wpool.tile([128, GW], f16, tag="wt")
                sg = wpool.tile([128, GW], f16, tag="sg")
                ug = wpool.tile([128, GW], f16, tag="ug")
                pg = wpool.tile([128, GW], f16, tag="pg")
                pg2 = wpool.tile([128, GW], f16, tag="pg2")
                t32 = wpool.tile([128, GW], f32, tag="t32")
                ag = xg                      # X dead after L0 i1
                gg = yz[:, 0:GW]             # yz dead after L1 i1
                gg2 = yz[:, GW:2 * GW]
                og = ug                      # U dead after Hp

                nc.sync.dma_start(xg[:], f_in[gi])
                nc.sync.dma_start(wog[:], wo_in[gi])
                yield

                def sl(tile_, j):
                    return tile_[:, j * N:(j + 1) * N]

                def mmq(out_ap, statT_ap, mov_ap):
                    for t in range(2):
                        ps = slice(64 * t, 64 * t + 64)
                        nc.tensor.matmul(out_ap[ps], statT_ap[ps], mov_ap[ps])

                def prod(pool, statT, mov, wide=False, tag="pT"):
                    w = 2 * GW if wide else GW
                    p = pool.tile([128, w], f32, tag=tag)
                    for j in range(GANG):
                        if wide:
                            mmq(p[:, 2 * N * j:2 * N * (j + 1)],
                                sl(statT, j), mov(j))
                        else:
                            mmq(sl(p, j), sl(statT, j), mov(j))
                    return p

                def yzv(tile_):
                    return tile_[:].rearrange("p (j d) -> p j d", d=2 * N)

                def Ysl(tile_, j):
                    return tile_[:, 2 * N * j: 2 * N * j + N]

                def Zsl(tile_, j):
                    return tile_[:, 2 * N * j + N: 2 * N * j + 2 * N]

                def YZsl(tile_, j):
                    return tile_[:, 2 * N * j: 2 * N * (j + 1)]

                # ---- level 0 ----
                # i1: W = a0 I + b0 X; Y1 = W@X; Z1 = W
                nc.vector.scalar_tensor_tensor(
                    wt[:], xg[:], float(LEV0[0][1]), cblk(0), mult, add)
                pY = prod(psA, wt, lambda j: sl(xg, j))
                Yv = yzv(yz)[:, :, 0:N]
                Zv = yzv(yz)[:, :, N:2 * N]
                nc.scalar.activation(Yv, pY[:].rearrange("p (j d) -> p j d",
                                                         d=N), CopyF)
                nc.gpsimd.tensor_copy(Zv, wt[:].rearrange("p (j d) -> p j d",
                                                          d=N))
                yield

                def ns_iter(src, dst, k, lev, cb, last=False):
                    # T = Z@Y ; W = a I + b T ; [Y'|Z'] = W @ [Y|Z]
                    b = lev[k][1]
                    pT = psA.tile([128, GW], f32, tag="pT")
                    for j in range(GANG):
                        mmq(sl(pT, j), Zsl(src, j), Ysl(src, j))
                    nc.vector.scalar_tensor_tensor(
                        wt[:], pT[:], float(b), cblk(cb), mult, add)
                    pYZ = psB.tile([128, 2 * GW], f32, tag="pYZ")
                    for j in range(GANG):
                        mmq(pYZ[:, 2 * N * j:2 * N * (j + 1)], sl(wt, j),
                            YZsl(src, j))
                    pv = pYZ[:].rearrange("p (j d) -> p j d", d=2 * N)
                    if last:
                        # zb = beta*Z' ; S = alpha*Y' - zb
                        nc.scalar.activation(ug[:].rearrange(
                            "p (j d) -> p j d", d=N), pv[:, :, N:2 * N],
                            CopyF, scale=float(BETA))
                        nc.vector.scalar_tensor_tensor(
                            sg[:].rearrange("p (j d) -> p j d", d=N),
                            pv[:, :, 0:N], float(ALPHA),
                            ug[:].rearrange("p (j d) -> p j d", d=N),
                            mult, sub)
                    else:
                        nc.scalar.activation(dst[:], pYZ[:], CopyF)

                for k in range(1, len(LEV0)):
                    ns_iter(yz, yz, k, LEV0, k)
                    yield

                # ---- level 1 ----
                # i1: W = a I + b Y ; Ynew = W@Y ; Znew = W
                Y2v = yzv(yz2)[:, :, 0:N]
                Z2v = yzv(yz2)[:, :, N:2 * N]
                nc.vector.scalar_tensor_tensor(
                    wt[:].rearrange("p (j d) -> p j d", d=N), Yv,
                    float(LEV1[0][1]),
                    cblk(4)[:].rearrange("p (j d) -> p j d", d=N), mult, add)
                pY2 = psA.tile([128, GW], f32, tag="pT")
                for j in range(GANG):
                    mmq(sl(pY2, j), sl(wt, j), Ysl(yz, j))
                nc.scalar.activation(Y2v, pY2[:].rearrange(
                    "p (j d) -> p j d", d=N), CopyF)
                nc.gpsimd.tensor_copy(Z2v, wt[:].rearrange(
                    "p (j d) -> p j d", d=N))
                yield

                ns_iter(yz2, yz2, 1, LEV1, 5)
                yield
                ns_iter(yz2, yz2, 2, LEV1, 6, last=True)
                yield

                # ---- asinh 3 terms: P = (cf2 U + cf1) U + cf0.
                #      U' = cf2*U (folded into ACT copy scale);
                #      P2 = U' + cf1 I (fast TT); P0 = (P2@U')/cf2 + cf0 I
                pU = psA.tile([128, GW], f32, tag="pT")
                for j in range(GANG):
                    mmq(sl(pU, j), sl(sg, j), sl(sg, j))
                nc.scalar.activation(ug[:], pU[:], CopyF, scale=float(ASC[2]))
                yield
                nc.vector.tensor_tensor(pg[:], ug[:], cblk16(1), add)
                pP = psA.tile([128, GW], f32, tag="pT")
                for j in range(GANG):
                    mmq(sl(pP, j), sl(ug, j), sl(pg, j))
                nc.vector.scalar_tensor_tensor(
                    pg2[:], pP[:], float(1.0 / ASC[2]), cblk(7), mult, add)
                yield
                # Hp = S @ P0 ; A' = wo o (2 Hp + gamma I)/DEG  (DEG=4 folded)
                pH = psA.tile([128, GW], f32, tag="pT")
                for j in range(GANG):
                    mmq(sl(pH, j), sl(sg, j), sl(pg2, j))
                nc.vector.scalar_tensor_tensor(
                    t32[:], pH[:], float(16.0 / (2 ** NSQ) / 4.0), cblk(8),
                    mult, add)
                nc.gpsimd.tensor_tensor(ag[:], t32[:], wog[:], mult)
                yield

                # ---- exp Taylor-4 Horner on A' = A/4:
                #      G = I + 4A'(I + 2A'(I + (4/3)A'(I + A')))
                nc.vector.tensor_tensor(gg, ag[:], cI16, add)
                pG = psA.tile([128, GW], f32, tag="pT")
                for j in range(GANG):
                    mmq(sl(pG, j), sl(ag, j), gg[:, j * N:(j + 1) * N])
                nc.vector.scalar_tensor_tensor(
                    gg2, pG[:], float(4.0 / 3.0), cI, mult, add)
                pG2 = psA.tile([128, GW], f32, tag="pT")
                for j in range(GANG):
                    mmq(sl(pG2, j), sl(ag, j), gg2[:, j * N:(j + 1) * N])
                nc.vector.scalar_tensor_tensor(
                    gg, pG2[:], 2.0, cI, mult, add)
                yield
                pG3 = psA.tile([128, GW], f32, tag="pT")
                for j in range(GANG):
                    mmq(sl(pG3, j), sl(ag, j), gg[:, j * N:(j + 1) * N])
                nc.vector.scalar_tensor_tensor(
                    gg2, pG3[:], 4.0, cI, mult, add)

                # ---- 2 squarings ----
                pS1 = psA.tile([128, GW], f32, tag="pT")
                for j in range(GANG):
                    mmq(sl(pS1, j), gg2[:, j * N:(j + 1) * N], gg2[:, j * N:(j + 1) * N])
                nc.scalar.activation(gg, pS1[:], CopyF)
                yield
                pS2 = psA.tile([128, GW], f32, tag="pT")
                for j in range(GANG):
                    mmq(sl(pS2, j), gg[:, j * N:(j + 1) * N], gg[:, j * N:(j + 1) * N])
                nc.scalar.activation(og[:], pS2[:], CopyF)
                nc.sync.dma_start(out_d[gi], og[:])

            def run_interleaved(ngang_, width):
                gens = []
                nxt = 0
                while gens or nxt < ngang_:
                    if len(gens) < width and nxt < ngang_:
                        gens.append(gang_stages(nxt))
                        nxt += 1
                    done = []
                    for g in gens:
                        try:
                            next(g)
                        except StopIteration:
                            done.append(g)
                    for g in done:
                        gens.remove(g)

            run_interleaved(ngang, INTERLEAVE)

    nc.compile()
    return nc


_cached = {}


def _get_nc(ngang=NGANG):
    if ngang not in _cached:
        _cached[ngang] = build_nc(ngang)
    return _cached[ngang]


def _in_maps(f, weights):
    f16 = f[:, 0].astype(np.float16)
    w32 = weights.astype(np.float32)
    cst = _host_constants()
    cst16 = _host_constants16()
    in_maps = []
    for c in range(N_CORES):
        sl_ = slice(c * SHARD, (c + 1) * SHARD)
        wc = w32[sl_]
        wo = (wc[:, :, None] * wc[:, None, :]).astype(np.float16)
        in_maps.append({
            "f": _rearr(f16[sl_]),
            "wo": _rearr(wo),
            "cst": cst,
            "cst16": cst16,
        })
    return in_maps


def kernel(f: np.ndarray, weights: np.ndarray) -> np.ndarray:
    from concourse.bass_utils import run_bass_kernel_spmd

    assert f.shape == (B_TOTAL, 1, N, N) and weights.shape == (B_TOTAL, N)
    nc = _get_nc()
    res = run_bass_kernel_spmd(nc, _in_maps(f, weights),
                               core_ids=list(range(N_CORES)))
    out = np.empty((B_TOTAL, 1, N, N), np.float32)
    for c in range(N_CORES):
        out[c * SHARD:(c + 1) * SHARD, 0] = \
            _unrearr(res.results[c]["out"]).astype(np.float32)
    return out


def run_traced(f: np.ndarray, weights: np.ndarray):
    from concourse.bass_utils import run_bass_kernel_spmd

    nc = _get_nc()
    return run_bass_kernel_spmd(nc, _in_maps(f, weights),
                                core_ids=list(range(N_CORES)), trace=True)
